# revision 54
# baseline (speedup 1.0000x reference)
"""DimNet output block for Trainium2, distributed over 8 NeuronCores.

Strategy: edges are bucketed on the host by destination-node tile (128 nodes
per tile); nodes are sharded across the 8 cores (no collectives needed).
Each core:
  - streams its edge shard (x rows pre-permuted into tile-major layout),
  - g = rbf @ W_rbf      (PE, K=6 matmul, one 128-edge chunk at a time)
  - xe = g * x           (DVE, fused over groups of 4 chunks)
  - pooled^T[tile] += xe^T @ onehot(r)   (PE, PSUM accumulation per node tile)
  - MLP: h = pooled @ W_up; 3x silu(h@W+b); out^T = W_final^T @ h3^T
  - writes out^T [12, 5120] which the host transposes/concatenates.

All data-dependent scheduling constants (chunks per node tile) are computed on
the host from the actual indices at call time and baked into the program; all
8 cores run the same program (max-over-cores padding keeps it uniform).
"""

import math
from contextlib import ExitStack

import ml_dtypes
import numpy as np

BF16 = ml_dtypes.bfloat16

P = 128
NUM_RADIAL = 6
EMB = 128
OUT_EMB = 256
NUM_TARGETS = 12
N_CORES = 8
MULG = 4  # chunks fused into one DVE multiply (512-wide)


def _ceil_div(a, b):
    return -(-a // b)


# ---------------------------------------------------------------------------
# Host-side preparation: bucket edges by destination tile, build per-core
# arrays in the exact SBUF layouts the kernel consumes.
# ---------------------------------------------------------------------------

def prepare_inputs(x, rbf, idnb_i, n_nodes, n_cores=N_CORES):
    n_edges = x.shape[0]
    idx = np.asarray(idnb_i).astype(np.int64)

    n_tiles_total = _ceil_div(n_nodes, P)          # e.g. 313
    tiles_per_core = _ceil_div(n_tiles_total, n_cores)  # e.g. 40
    nodes_per_core = tiles_per_core * P            # e.g. 5120

    tile_g = idx >> 7                              # global tile id
    r_int = (idx & 127).astype(np.int64)           # node-within-tile

    order = np.lexsort((r_int, tile_g))            # by tile, then node
    counts = np.bincount(tile_g, minlength=n_cores * tiles_per_core)
    counts2 = counts.reshape(n_cores, tiles_per_core)
    # uniform chunk schedule across cores: max over cores, per tile slot
    chunks = _ceil_div(counts2.max(axis=0), P)     # [tiles_per_core]
    chunks = np.asarray(chunks, dtype=np.int64)
    cbase = np.zeros(tiles_per_core + 1, dtype=np.int64)
    cbase[1:] = np.cumsum(chunks)
    CH = int(cbase[-1])                            # chunks per core
    S = CH * P                                     # edge slots per core

    gstart = np.zeros(n_cores * tiles_per_core + 1, dtype=np.int64)
    gstart[1:] = np.cumsum(counts)

    x = np.ascontiguousarray(x, dtype=np.float32)
    rbf = np.ascontiguousarray(rbf, dtype=np.float32)

    x_sh = np.zeros((n_cores, P, S), dtype=np.float32)     # [p][c*128+f]
    rbf_sh = np.zeros((n_cores, NUM_RADIAL, S), dtype=np.float32)
    r_sh = np.zeros((n_cores, P, CH), dtype=np.float32)

    # per-tile identity-chunk budget B[t]: each node's first B edges go to
    # fixed slots (partition == node) so those chunks use a constant
    # identity matrix instead of a DVE-generated one-hot.
    fa = np.arange(P)
    degs = np.zeros((n_cores, tiles_per_core, P), dtype=np.int64)
    for c in range(n_cores):
        for t in range(tiles_per_core):
            g = c * tiles_per_core + t
            el = order[gstart[g]:gstart[g] + counts[g]]
            degs[c, t] = np.bincount(r_int[el], minlength=P)
    B = np.zeros(tiles_per_core, dtype=np.int64)
    for t in range(tiles_per_core):
        ct = int(chunks[t])
        for b in range(ct, -1, -1):
            ok = True
            for c in range(n_cores):
                ov = int(np.maximum(degs[c, t] - b, 0).sum())
                if b + _ceil_div(ov, P) > ct:
                    ok = False
                    break
            if ok:
                B[t] = b
                break

    for c in range(n_cores):
        for t in range(tiles_per_core):
            g = c * tiles_per_core + t
            n = counts[g]
            if n == 0:
                continue
            el = order[gstart[g]:gstart[g] + n]          # sorted by node r
            rr = r_int[el]
            d = degs[c, t]
            nstart = np.zeros(P + 1, dtype=np.int64)
            nstart[1:] = np.cumsum(d)
            occ = np.arange(n) - nstart[rr]              # rank within node
            bt = int(B[t])
            is_id = occ < bt
            cc = np.empty(n, dtype=np.int64)
            pp = np.empty(n, dtype=np.int64)
            cc[is_id] = occ[is_id]
            pp[is_id] = rr[is_id]
            ng = int((~is_id).sum())
            cc[~is_id] = bt + np.arange(ng) // P
            pp[~is_id] = np.arange(ng) % P
            col = (cbase[t] + cc) * P                    # x free-dim base
            # x layout: [partition pp, free (chunk,f)]
            x_sh[c, pp[:, None], col[:, None] + fa[None, :]] = x[el]
            rbf_sh[c, :, col + pp] = rbf[el]  # adv. index moves axis to front
            r_sh[c, pp, cbase[t] + cc] = rr

    meta = dict(
        tiles_per_core=tiles_per_core,
        nodes_per_core=nodes_per_core,
        chunks=[int(v) for v in chunks],
        ident=[int(v) for v in B],
        CH=CH,
        S=S,
    )
    return x_sh, rbf_sh.astype(BF16), r_sh, meta


# ---------------------------------------------------------------------------
# Device program
# ---------------------------------------------------------------------------

def build(meta, reps=1):
    import concourse.bacc as bacc
    import concourse.mybir as mybir
    import concourse.tile as tile

    f32 = mybir.dt.float32
    f32r = mybir.dt.float32r
    bf16 = mybir.dt.bfloat16
    chunks = meta["chunks"]
    ident = meta.get("ident", [0] * len(chunks))
    CH = meta["CH"]
    S = meta["S"]
    n_tiles = meta["tiles_per_core"]
    nodes = meta["nodes_per_core"]
    NL = 3  # number of MLP layers

    nc = bacc.Bacc("TRN2", target_bir_lowering=False, debug=False,
                   num_devices=N_CORES)

    x_d = nc.dram_tensor("x_sh", [P, S], f32, kind="ExternalInput").ap()
    rbf_d = nc.dram_tensor("rbf_sh", [NUM_RADIAL, S], bf16,
                           kind="ExternalInput").ap()
    r_d = nc.dram_tensor("r_sh", [P, CH], f32, kind="ExternalInput").ap()
    wrbf_d = nc.dram_tensor("W_rbf", [NUM_RADIAL, EMB], bf16,
                            kind="ExternalInput").ap()
    wup_d = nc.dram_tensor("W_up", [EMB, OUT_EMB], f32r,
                           kind="ExternalInput").ap()
    wmlp_d = nc.dram_tensor("W_mlp", [NL, OUT_EMB, OUT_EMB], f32r,
                            kind="ExternalInput").ap()
    b_d = nc.dram_tensor("b_h", [P, 2 * NL], f32, kind="ExternalInput").ap()
    wf_d = nc.dram_tensor("W_final", [OUT_EMB, NUM_TARGETS], f32r,
                          kind="ExternalInput").ap()
    iota_d = nc.dram_tensor("iota_h", [P, P], bf16, kind="ExternalInput").ap()
    ident_d = nc.dram_tensor("ident_h", [P, P], bf16, kind="ExternalInput").ap()
    out_d = nc.dram_tensor("outT", [NUM_TARGETS, nodes], f32,
                           kind="ExternalOutput").ap()

    with tile.TileContext(nc) as tc, ExitStack() as ctx:
        const = ctx.enter_context(tc.tile_pool(name="const", bufs=1))
        xpool = ctx.enter_context(tc.tile_pool(name="xpool", bufs=5))
        rbfpool = ctx.enter_context(tc.tile_pool(name="rbfpool", bufs=5))
        ohpool = ctx.enter_context(tc.tile_pool(name="ohpool", bufs=48))
        xepool = ctx.enter_context(tc.tile_pool(name="xepool", bufs=6))
        hpool = ctx.enter_context(tc.tile_pool(name="hpool", bufs=6))
        opool = ctx.enter_context(tc.tile_pool(name="opool", bufs=1))
        gps_pool = ctx.enter_context(
            tc.tile_pool(name="gps", bufs=4, space="PSUM"))
        accps_pool = ctx.enter_context(
            tc.tile_pool(name="accps", bufs=2, space="PSUM"))
        mlpps_pool = ctx.enter_context(
            tc.tile_pool(name="mlpps", bufs=2, space="PSUM"))

        # ---- constants into SBUF ----
        wrbf_sb = const.tile([NUM_RADIAL, EMB], bf16)
        nc.sync.dma_start(wrbf_sb[:], wrbf_d[:, :])
        wup_sb = const.tile([P, OUT_EMB], f32r)
        nc.sync.dma_start(wup_sb[:], wup_d[:, :])
        wm_sb = const.tile([P, NL, 2, OUT_EMB], f32r)
        for i in range(NL):
            for kh in range(2):
                nc.sync.dma_start(wm_sb[:, i, kh, :],
                                  wmlp_d[i, kh * P:(kh + 1) * P, :])
        b_sb = const.tile([P, 2 * NL], f32)
        nc.sync.dma_start(b_sb[:], b_d[:, :])
        wf_sb = const.tile([P, 2, NUM_TARGETS], f32r)
        for kh in range(2):
            nc.sync.dma_start(wf_sb[:, kh, :], wf_d[kh * P:(kh + 1) * P, :])
        iota_sb = const.tile([P, P], bf16)
        nc.sync.dma_start(iota_sb[:], iota_d[:, :])
        ident_sb = const.tile([P, P], bf16)
        nc.sync.dma_start(ident_sb[:], ident_d[:, :])
        r_sb = const.tile([P, CH], f32)
        nc.sync.dma_start(r_sb[:], r_d[:, :])

        pooled_sb = opool.tile([P, nodes], f32r)   # pooled^T, persistent
        outT_sb = opool.tile([NUM_TARGETS, nodes], f32)

        NG = min(512, nodes)
        Sigmoid = mybir.ActivationFunctionType.Sigmoid
        Identity = mybir.ActivationFunctionType.Identity

        # ---- binning phase ----
        def binning():
          for t in range(n_tiles):
            ch = int(chunks[t])
            bt = int(ident[t])
            if ch == 0:
                nc.vector.memset(pooled_sb[:, t * P:(t + 1) * P], 0.0)
                continue
            cb = int(sum(chunks[:t]))
            x_t = xpool.tile([P, ch * P], f32, tag="x")
            nc.sync.dma_start(x_t[:], x_d[:, cb * P:(cb + ch) * P])
            rbf_t = rbfpool.tile([NUM_RADIAL, ch * P], bf16, tag="rbf")
            nc.sync.dma_start(rbf_t[:], rbf_d[:, cb * P:(cb + ch) * P])

            acc_ps = accps_pool.tile([P, P], f32, tag="acc")
            n_groups = _ceil_div(ch, MULG)
            pend = None  # software pipeline: bin-MMs lag one group behind

            def flush(p):
                xe_p, ohs_p, p0, p1 = p
                for j in range(p0, p1):
                    nc.tensor.matmul(
                        out=acc_ps[:],
                        lhsT=xe_p[:, (j - p0) * P:(j - p0 + 1) * P],
                        rhs=ohs_p[j - p0][:],
                        start=(j == 0), stop=(j == ch - 1),
                    )

            for gi in range(n_groups):
                c0 = gi * MULG
                c1 = min(c0 + MULG, ch)
                gw = (c1 - c0) * P
                g_ps = gps_pool.tile([P, MULG * P], f32, tag="gps")
                onehots = []
                for j in range(c0, c1):
                    if j < bt:
                        onehots.append(ident_sb)
                    else:
                        oh_t = ohpool.tile([P, P], bf16, tag="oh")
                        nc.vector.tensor_scalar(
                            out=oh_t[:],
                            in0=iota_sb[:],
                            scalar1=r_sb[:, cb + j:cb + j + 1],
                            scalar2=None,
                            op0=mybir.AluOpType.is_equal,
                        )
                        onehots.append(oh_t)
                    nc.tensor.matmul(
                        out=g_ps[:, (j - c0) * P:(j - c0 + 1) * P],
                        lhsT=rbf_t[:, j * P:(j + 1) * P],
                        rhs=wrbf_sb[:],
                        start=True, stop=True,
                    )
                xe_t = xepool.tile([P, MULG * P], bf16, tag="xe")
                nc.vector.tensor_tensor(
                    out=xe_t[:, :gw],
                    in0=g_ps[:, :gw],
                    in1=x_t[:, c0 * P:c0 * P + gw],
                    op=mybir.AluOpType.mult,
                )
                if pend is not None:
                    flush(pend)
                pend = (xe_t, onehots, c0, c1)
            flush(pend)
            nc.scalar.copy(pooled_sb[:, t * P:(t + 1) * P], acc_ps[:])
            if (t + 1) % 4 == 0:
                mlp_group((t - 3) * P)
            elif t == n_tiles - 1:
                mlp_group((n_tiles - n_tiles % 4) * P)

        # ---- MLP (nodes on the moving free dim, 512 at a time) ----
        def mlp_group(n0):
            if n0 >= nodes:
                return
            w = min(NG, nodes - n0)
            rhs = pooled_sb[:, n0:n0 + w]
            hs = None
            for i in range(NL):
                new_hs = []
                for oh in range(2):
                    ps = mlpps_pool.tile([P, NG], f32, tag="mlp")
                    if i == 0:
                        nc.tensor.matmul(out=ps[:, :w],
                                         lhsT=wup_sb[:, oh * P:(oh + 1) * P],
                                         rhs=rhs, start=True, stop=True)
                    else:
                        nc.tensor.matmul(out=ps[:, :w],
                                         lhsT=wm_sb[:, i, 0, oh * P:(oh + 1) * P],
                                         rhs=hs[0][:, :w],
                                         start=True, stop=False)
                        nc.tensor.matmul(out=ps[:, :w],
                                         lhsT=wm_sb[:, i, 1, oh * P:(oh + 1) * P],
                                         rhs=hs[1][:, :w],
                                         start=False, stop=True)
                    bias_ap = b_sb[:, 2 * i + oh:2 * i + oh + 1]
                    s_sb = hpool.tile([P, NG], f32, tag="s")
                    nc.scalar.activation(s_sb[:, :w], ps[:, :w], Sigmoid,
                                         bias=bias_ap)
                    h_sb = hpool.tile([P, NG], f32r, tag="h")
                    nc.vector.scalar_tensor_tensor(
                        out=h_sb[:, :w], in0=ps[:, :w], scalar=bias_ap,
                        in1=s_sb[:, :w], op0=mybir.AluOpType.add,
                        op1=mybir.AluOpType.mult)
                    new_hs.append(h_sb)
                hs = new_hs
            ps_o = mlpps_pool.tile([P, NG], f32, tag="mlp",
                                   name="ps_o")[:NUM_TARGETS, :]
            nc.tensor.matmul(out=ps_o[:, :w], lhsT=wf_sb[:, 0, :],
                             rhs=hs[0][:, :w],
                             start=True, stop=False)
            nc.tensor.matmul(out=ps_o[:, :w], lhsT=wf_sb[:, 1, :],
                             rhs=hs[1][:, :w],
                             start=False, stop=True)
            nc.scalar.copy(outT_sb[:, n0:n0 + w], ps_o[:, :w])

        def body():
            binning()
            nc.sync.dma_start(out_d[:, :], outT_sb[:])

        if reps == 1:
            body()
        else:
            hints = (mybir.EngineType.PE, mybir.EngineType.DVE,
                     mybir.EngineType.Activation, mybir.EngineType.Pool,
                     mybir.EngineType.SP)
            with tc.For_i(0, reps, 1, hint_engines=hints):
                body()

    nc.compile()
    return nc


# ---------------------------------------------------------------------------
# PJRT runner with device-resident inputs + repeat timing
# ---------------------------------------------------------------------------

def _run_spmd_pjrt(nc, in_maps, n_cores, timing_iters=0):
    import time as _time

    import jax
    from jax.experimental.shard_map import shard_map
    from jax.sharding import Mesh, NamedSharding, PartitionSpec

    from concourse import bass2jax, mybir

    bass2jax.install_neuronx_cc_hook()
    partition_name = (nc.partition_id_tensor.name
                      if nc.partition_id_tensor else None)
    in_names, out_names, out_avals, zero_outs = [], [], [], []
    for alloc in nc.m.functions[0].allocations:
        if not isinstance(alloc, mybir.MemoryLocationSet):
            continue
        name = alloc.memorylocations[0].name
        if alloc.kind == "ExternalInput":
            if name != partition_name:
                in_names.append(name)
        elif alloc.kind == "ExternalOutput":
            shape = tuple(alloc.tensor_shape)
            dtype = mybir.dt.np(alloc.dtype)
            out_names.append(name)
            out_avals.append(jax.core.ShapedArray(shape, dtype))
            zero_outs.append(np.zeros(shape, dtype))
    n_params = len(in_names)
    n_outs = len(out_avals)
    all_names = list(in_names) + list(out_names)
    if partition_name is not None:
        all_names.append(partition_name)
    donate = tuple(range(n_params, n_params + n_outs))

    def _body(*args):
        operands = list(args)
        if partition_name is not None:
            operands.append(bass2jax.partition_id_tensor())
        outs = bass2jax._bass_exec_p.bind(
            *operands,
            out_avals=tuple(out_avals),
            in_names=tuple(all_names),
            out_names=tuple(out_names),
            lowering_input_output_aliases=(),
            sim_require_finite=True,
            sim_require_nnan=True,
            nc=nc,
        )
        return tuple(outs)

    devices = jax.devices()[:n_cores]
    mesh = Mesh(np.asarray(devices), ("core",))
    in_specs = (PartitionSpec("core"),) * (n_params + n_outs)
    out_specs = (PartitionSpec("core"),) * len(out_names)
    fn = jax.jit(
        shard_map(_body, mesh=mesh, in_specs=in_specs, out_specs=out_specs,
                  check_rep=False),
        donate_argnums=donate, keep_unused=True)
    sharding = NamedSharding(mesh, PartitionSpec("core"))
    concat_in = [
        jax.device_put(
            np.concatenate([np.asarray(in_maps[c][nm]) for c in range(n_cores)],
                           axis=0), sharding)
        for nm in in_names
    ]

    def zeros():
        zs = [jax.device_put(
            np.zeros((n_cores * z.shape[0], *z.shape[1:]), z.dtype), sharding)
            for z in zero_outs]
        for z in zs:
            z.block_until_ready()
        return zs

    out_arrs = fn(*concat_in, *zeros())
    for o in out_arrs:
        o.block_until_ready()
    times = []
    for _ in range(timing_iters):
        zs = zeros()
        t0 = _time.perf_counter()
        outs2 = fn(*concat_in, *zs)
        for o in outs2:
            o.block_until_ready()
        times.append(_time.perf_counter() - t0)
    results = [
        {name: np.asarray(out_arrs[i]).reshape(n_cores, *out_avals[i].shape)[c]
         for i, name in enumerate(out_names)}
        for c in range(n_cores)
    ]
    return results, times


# ---------------------------------------------------------------------------
# Entry point
# ---------------------------------------------------------------------------

_BUILD_CACHE = {}


def make_in_maps(x_sh, rbf_sh, r_sh, W_rbf, W_up, W_mlp, b_mlp, W_final):
    W_rbf = np.ascontiguousarray(W_rbf, dtype=np.float32).astype(BF16)
    # fold the bias-free up-projection into the first MLP layer (fp64 host
    # precompute): h1 = silu(pooled @ (W_up @ W_mlp[0]) + b0)
    W_up = (np.asarray(W_up, np.float64) @ np.asarray(W_mlp[0], np.float64)
            ).astype(np.float32)
    W_mlp = np.ascontiguousarray(W_mlp, dtype=np.float32)
    W_final = np.ascontiguousarray(W_final, dtype=np.float32)
    b_mlp = np.asarray(b_mlp, dtype=np.float32)
    NL = W_mlp.shape[0]
    b_h = np.zeros((P, 2 * NL), dtype=np.float32)
    for i in range(NL):
        for oh in range(2):
            b_h[:, 2 * i + oh] = b_mlp[i, oh * P:(oh + 1) * P]
    iota_h = np.broadcast_to(
        np.arange(P, dtype=np.float32)[None, :], (P, P)).astype(BF16)
    ident_h = np.eye(P, dtype=np.float32).astype(BF16)

    in_maps = []
    for c in range(N_CORES):
        in_maps.append({
            "x_sh": x_sh[c],
            "rbf_sh": rbf_sh[c],
            "r_sh": r_sh[c],
            "W_rbf": W_rbf,
            "W_up": W_up,
            "W_mlp": W_mlp,
            "b_h": b_h,
            "W_final": W_final,
            "iota_h": iota_h,
            "ident_h": ident_h,
        })
    return in_maps


def kernel(n_atoms, x, rbf, idnb_i, W_rbf, W_up, W_mlp, b_mlp, W_final,
           timing_iters=0, reps=1, run_kwargs=None):
    n_nodes = n_atoms.shape[0]
    x_sh, rbf_sh, r_sh, meta = prepare_inputs(x, rbf, idnb_i, n_nodes)

    key = (n_nodes, tuple(meta["chunks"]), tuple(meta["ident"]), reps)
    if key not in _BUILD_CACHE:
        _BUILD_CACHE[key] = build(meta, reps=reps)
    nc = _BUILD_CACHE[key]

    in_maps = make_in_maps(x_sh, rbf_sh, r_sh, W_rbf, W_up, W_mlp, b_mlp,
                           W_final)
    try:
        results, times = _run_spmd_pjrt(nc, in_maps, N_CORES,
                                        timing_iters=timing_iters)
    except Exception:
        from concourse.bass_utils import run_bass_kernel_spmd
        res = run_bass_kernel_spmd(nc, in_maps, core_ids=list(range(N_CORES)))
        results, times = res.results, []
    outs = [results[c]["outT"].T for c in range(N_CORES)]
    full = np.concatenate(outs, axis=0)[:n_nodes]
    kernel.last_times = times
    return full.astype(np.float32)


# revision 56
# speedup vs baseline: 1.1226x; 1.1226x over previous
"""DimNet output block for Trainium2, distributed over 8 NeuronCores.

Strategy: edges are bucketed on the host by destination-node tile (128 nodes
per tile); nodes are sharded across the 8 cores (no collectives needed).
Each core:
  - streams its edge shard (x rows pre-permuted into tile-major layout),
  - g = rbf @ W_rbf      (PE, K=6 matmul, one 128-edge chunk at a time)
  - xe = g * x           (DVE, fused over groups of 4 chunks)
  - pooled^T[tile] += xe^T @ onehot(r)   (PE, PSUM accumulation per node tile)
  - MLP: h = pooled @ W_up; 3x silu(h@W+b); out^T = W_final^T @ h3^T
  - writes out^T [12, 5120] which the host transposes/concatenates.

All data-dependent scheduling constants (chunks per node tile) are computed on
the host from the actual indices at call time and baked into the program; all
8 cores run the same program (max-over-cores padding keeps it uniform).
"""

import math
from contextlib import ExitStack

import ml_dtypes
import numpy as np

BF16 = ml_dtypes.bfloat16

P = 128
NUM_RADIAL = 6
EMB = 128
OUT_EMB = 256
NUM_TARGETS = 12
N_CORES = 8
MULG = 4  # chunks fused into one DVE multiply (512-wide)


def _ceil_div(a, b):
    return -(-a // b)


# ---------------------------------------------------------------------------
# Host-side preparation: bucket edges by destination tile, build per-core
# arrays in the exact SBUF layouts the kernel consumes.
# ---------------------------------------------------------------------------

def prepare_inputs(x, rbf, idnb_i, n_nodes, n_cores=N_CORES):
    n_edges = x.shape[0]
    idx = np.asarray(idnb_i).astype(np.int64)

    n_tiles_total = _ceil_div(n_nodes, P)          # e.g. 313
    tiles_per_core = _ceil_div(n_tiles_total, n_cores)  # e.g. 40
    nodes_per_core = tiles_per_core * P            # e.g. 5120

    tile_g = idx >> 7                              # global tile id
    r_int = (idx & 127).astype(np.int64)           # node-within-tile

    order = np.lexsort((r_int, tile_g))            # by tile, then node
    counts = np.bincount(tile_g, minlength=n_cores * tiles_per_core)
    # Balance: assign tiles to (core, slot) so that similar-sized tiles
    # share a slot -- the per-slot chunk count is the max over the 8 cores,
    # so grouping by size minimizes padding. asgn[c, t] = global tile id.
    ranks = np.argsort(-counts)                    # descending by edge count
    asgn = ranks.reshape(tiles_per_core, n_cores).T  # [n_cores, slots]
    counts2 = counts[asgn]                         # [n_cores, slots]
    chunks = _ceil_div(counts2.max(axis=0), P)     # per-slot chunk count
    chunks = np.asarray(chunks, dtype=np.int64)
    cbase = np.zeros(tiles_per_core + 1, dtype=np.int64)
    cbase[1:] = np.cumsum(chunks)
    CH = int(cbase[-1])                            # chunks per core
    S = CH * P                                     # edge slots per core

    gstart = np.zeros(n_cores * tiles_per_core + 1, dtype=np.int64)
    gstart[1:] = np.cumsum(counts)

    x = np.ascontiguousarray(x, dtype=np.float32)
    rbf = np.ascontiguousarray(rbf, dtype=np.float32)

    x_sh = np.zeros((n_cores, P, S), dtype=np.float32)     # [p][c*128+f]
    rbf_sh = np.zeros((n_cores, NUM_RADIAL, S), dtype=np.float32)
    r_sh = np.zeros((n_cores, P, CH), dtype=np.float32)

    # per-tile identity-chunk budget B[t]: each node's first B edges go to
    # fixed slots (partition == node) so those chunks use a constant
    # identity matrix instead of a DVE-generated one-hot.
    fa = np.arange(P)
    degs = np.zeros((n_cores, tiles_per_core, P), dtype=np.int64)
    for c in range(n_cores):
        for t in range(tiles_per_core):
            g = int(asgn[c, t])
            el = order[gstart[g]:gstart[g] + counts[g]]
            degs[c, t] = np.bincount(r_int[el], minlength=P)
    B = np.zeros(tiles_per_core, dtype=np.int64)
    for t in range(tiles_per_core):
        ct = int(chunks[t])
        for b in range(ct, -1, -1):
            ok = True
            for c in range(n_cores):
                ov = int(np.maximum(degs[c, t] - b, 0).sum())
                if b + _ceil_div(ov, P) > ct:
                    ok = False
                    break
            if ok:
                B[t] = b
                break

    for c in range(n_cores):
        for t in range(tiles_per_core):
            g = int(asgn[c, t])
            n = counts[g]
            if n == 0:
                continue
            el = order[gstart[g]:gstart[g] + n]          # sorted by node r
            rr = r_int[el]
            d = degs[c, t]
            nstart = np.zeros(P + 1, dtype=np.int64)
            nstart[1:] = np.cumsum(d)
            occ = np.arange(n) - nstart[rr]              # rank within node
            bt = int(B[t])
            is_id = occ < bt
            cc = np.empty(n, dtype=np.int64)
            pp = np.empty(n, dtype=np.int64)
            cc[is_id] = occ[is_id]
            pp[is_id] = rr[is_id]
            ng = int((~is_id).sum())
            cc[~is_id] = bt + np.arange(ng) // P
            pp[~is_id] = np.arange(ng) % P
            col = (cbase[t] + cc) * P                    # x free-dim base
            # x layout: [partition pp, free (chunk,f)]
            x_sh[c, pp[:, None], col[:, None] + fa[None, :]] = x[el]
            rbf_sh[c, :, col + pp] = rbf[el]  # adv. index moves axis to front
            r_sh[c, pp, cbase[t] + cc] = rr

    meta = dict(
        tiles_per_core=tiles_per_core,
        nodes_per_core=nodes_per_core,
        chunks=[int(v) for v in chunks],
        ident=[int(v) for v in B],
        CH=CH,
        S=S,
        asgn=asgn.tolist(),
    )
    return x_sh, rbf_sh.astype(BF16), r_sh, meta


# ---------------------------------------------------------------------------
# Device program
# ---------------------------------------------------------------------------

def build(meta, reps=1):
    import concourse.bacc as bacc
    import concourse.mybir as mybir
    import concourse.tile as tile

    f32 = mybir.dt.float32
    f32r = mybir.dt.float32r
    bf16 = mybir.dt.bfloat16
    chunks = meta["chunks"]
    ident = meta.get("ident", [0] * len(chunks))
    CH = meta["CH"]
    S = meta["S"]
    n_tiles = meta["tiles_per_core"]
    nodes = meta["nodes_per_core"]
    NL = 3  # number of MLP layers

    nc = bacc.Bacc("TRN2", target_bir_lowering=False, debug=False,
                   num_devices=N_CORES)

    x_d = nc.dram_tensor("x_sh", [P, S], f32, kind="ExternalInput").ap()
    rbf_d = nc.dram_tensor("rbf_sh", [NUM_RADIAL, S], bf16,
                           kind="ExternalInput").ap()
    r_d = nc.dram_tensor("r_sh", [P, CH], f32, kind="ExternalInput").ap()
    wrbf_d = nc.dram_tensor("W_rbf", [NUM_RADIAL, EMB], bf16,
                            kind="ExternalInput").ap()
    wup_d = nc.dram_tensor("W_up", [EMB, OUT_EMB], f32r,
                           kind="ExternalInput").ap()
    wmlp_d = nc.dram_tensor("W_mlp", [NL, OUT_EMB, OUT_EMB], f32r,
                            kind="ExternalInput").ap()
    b_d = nc.dram_tensor("b_h", [P, 2 * NL], f32, kind="ExternalInput").ap()
    wf_d = nc.dram_tensor("W_final", [OUT_EMB, NUM_TARGETS], f32r,
                          kind="ExternalInput").ap()
    iota_d = nc.dram_tensor("iota_h", [P, P], bf16, kind="ExternalInput").ap()
    ident_d = nc.dram_tensor("ident_h", [P, P], bf16, kind="ExternalInput").ap()
    out_d = nc.dram_tensor("outT", [NUM_TARGETS, nodes], f32,
                           kind="ExternalOutput").ap()

    with tile.TileContext(nc) as tc, ExitStack() as ctx:
        const = ctx.enter_context(tc.tile_pool(name="const", bufs=1))
        xpool = ctx.enter_context(tc.tile_pool(name="xpool", bufs=5))
        rbfpool = ctx.enter_context(tc.tile_pool(name="rbfpool", bufs=5))
        ohpool = ctx.enter_context(tc.tile_pool(name="ohpool", bufs=4 * MULG))
        xepool = ctx.enter_context(tc.tile_pool(name="xepool", bufs=6))
        hpool = ctx.enter_context(tc.tile_pool(name="hpool", bufs=6))
        opool = ctx.enter_context(tc.tile_pool(name="opool", bufs=1))
        gps_pool = ctx.enter_context(
            tc.tile_pool(name="gps", bufs=4, space="PSUM"))
        accps_pool = ctx.enter_context(
            tc.tile_pool(name="accps", bufs=2, space="PSUM"))
        mlpps_pool = ctx.enter_context(
            tc.tile_pool(name="mlpps", bufs=2, space="PSUM"))

        # ---- constants into SBUF ----
        wrbf_sb = const.tile([NUM_RADIAL, EMB], bf16)
        nc.sync.dma_start(wrbf_sb[:], wrbf_d[:, :])
        wup_sb = const.tile([P, OUT_EMB], f32r)
        nc.sync.dma_start(wup_sb[:], wup_d[:, :])
        wm_sb = const.tile([P, NL, 2, OUT_EMB], f32r)
        for i in range(NL):
            for kh in range(2):
                nc.sync.dma_start(wm_sb[:, i, kh, :],
                                  wmlp_d[i, kh * P:(kh + 1) * P, :])
        b_sb = const.tile([P, 2 * NL], f32)
        nc.sync.dma_start(b_sb[:], b_d[:, :])
        wf_sb = const.tile([P, 2, NUM_TARGETS], f32r)
        for kh in range(2):
            nc.sync.dma_start(wf_sb[:, kh, :], wf_d[kh * P:(kh + 1) * P, :])
        iota_sb = const.tile([P, P], bf16)
        nc.sync.dma_start(iota_sb[:], iota_d[:, :])
        ident_sb = const.tile([P, P], bf16)
        nc.sync.dma_start(ident_sb[:], ident_d[:, :])
        r_sb = const.tile([P, CH], f32)
        nc.sync.dma_start(r_sb[:], r_d[:, :])

        pooled_sb = opool.tile([P, nodes], f32r)   # pooled^T, persistent
        outT_sb = opool.tile([NUM_TARGETS, nodes], f32)

        NG = min(512, nodes)
        Sigmoid = mybir.ActivationFunctionType.Sigmoid
        Identity = mybir.ActivationFunctionType.Identity

        # ---- binning phase ----
        def binning():
          for t in range(n_tiles):
            ch = int(chunks[t])
            bt = int(ident[t])
            if ch == 0:
                nc.vector.memset(pooled_sb[:, t * P:(t + 1) * P], 0.0)
                continue
            cb = int(sum(chunks[:t]))
            x_t = xpool.tile([P, ch * P], f32, tag="x")
            nc.sync.dma_start(x_t[:], x_d[:, cb * P:(cb + ch) * P])
            rbf_t = rbfpool.tile([NUM_RADIAL, ch * P], bf16, tag="rbf")
            nc.sync.dma_start(rbf_t[:], rbf_d[:, cb * P:(cb + ch) * P])

            acc_ps = accps_pool.tile([P, P], f32, tag="acc")
            n_groups = _ceil_div(ch, MULG)
            pend = None  # software pipeline: bin-MMs lag one group behind

            def flush(p):
                xe_p, ohs_p, p0, p1 = p
                for j in range(p0, p1):
                    nc.tensor.matmul(
                        out=acc_ps[:],
                        lhsT=xe_p[:, (j - p0) * P:(j - p0 + 1) * P],
                        rhs=ohs_p[j - p0][:],
                        start=(j == 0), stop=(j == ch - 1),
                    )

            for gi in range(n_groups):
                c0 = gi * MULG
                c1 = min(c0 + MULG, ch)
                gw = (c1 - c0) * P
                g_ps = gps_pool.tile([P, MULG * P], f32, tag="gps")
                onehots = []
                for j in range(c0, c1):
                    if j < bt:
                        onehots.append(ident_sb)
                    else:
                        oh_t = ohpool.tile([P, P], bf16, tag="oh")
                        nc.vector.tensor_scalar(
                            out=oh_t[:],
                            in0=iota_sb[:],
                            scalar1=r_sb[:, cb + j:cb + j + 1],
                            scalar2=None,
                            op0=mybir.AluOpType.is_equal,
                        )
                        onehots.append(oh_t)
                    nc.tensor.matmul(
                        out=g_ps[:, (j - c0) * P:(j - c0 + 1) * P],
                        lhsT=rbf_t[:, j * P:(j + 1) * P],
                        rhs=wrbf_sb[:],
                        start=True, stop=True,
                    )
                xe_t = xepool.tile([P, MULG * P], bf16, tag="xe")
                nc.vector.tensor_tensor(
                    out=xe_t[:, :gw],
                    in0=g_ps[:, :gw],
                    in1=x_t[:, c0 * P:c0 * P + gw],
                    op=mybir.AluOpType.mult,
                )
                if pend is not None:
                    flush(pend)
                pend = (xe_t, onehots, c0, c1)
            flush(pend)
            nc.scalar.copy(pooled_sb[:, t * P:(t + 1) * P], acc_ps[:])
            if (t + 1) % 4 == 0:
                mlp_group((t - 3) * P)
            elif t == n_tiles - 1:
                mlp_group((n_tiles - n_tiles % 4) * P)

        # ---- MLP (nodes on the moving free dim, 512 at a time) ----
        def mlp_group(n0):
            if n0 >= nodes:
                return
            w = min(NG, nodes - n0)
            rhs = pooled_sb[:, n0:n0 + w]
            hs = None
            for i in range(NL):
                new_hs = []
                for oh in range(2):
                    ps = mlpps_pool.tile([P, NG], f32, tag="mlp")
                    if i == 0:
                        nc.tensor.matmul(out=ps[:, :w],
                                         lhsT=wup_sb[:, oh * P:(oh + 1) * P],
                                         rhs=rhs, start=True, stop=True)
                    else:
                        nc.tensor.matmul(out=ps[:, :w],
                                         lhsT=wm_sb[:, i, 0, oh * P:(oh + 1) * P],
                                         rhs=hs[0][:, :w],
                                         start=True, stop=False)
                        nc.tensor.matmul(out=ps[:, :w],
                                         lhsT=wm_sb[:, i, 1, oh * P:(oh + 1) * P],
                                         rhs=hs[1][:, :w],
                                         start=False, stop=True)
                    bias_ap = b_sb[:, 2 * i + oh:2 * i + oh + 1]
                    s_sb = hpool.tile([P, NG], f32, tag="s")
                    nc.scalar.activation(s_sb[:, :w], ps[:, :w], Sigmoid,
                                         bias=bias_ap)
                    h_sb = hpool.tile([P, NG], f32r, tag="h")
                    nc.vector.scalar_tensor_tensor(
                        out=h_sb[:, :w], in0=ps[:, :w], scalar=bias_ap,
                        in1=s_sb[:, :w], op0=mybir.AluOpType.add,
                        op1=mybir.AluOpType.mult)
                    new_hs.append(h_sb)
                hs = new_hs
            ps_o = mlpps_pool.tile([P, NG], f32, tag="mlp",
                                   name="ps_o")[:NUM_TARGETS, :]
            nc.tensor.matmul(out=ps_o[:, :w], lhsT=wf_sb[:, 0, :],
                             rhs=hs[0][:, :w],
                             start=True, stop=False)
            nc.tensor.matmul(out=ps_o[:, :w], lhsT=wf_sb[:, 1, :],
                             rhs=hs[1][:, :w],
                             start=False, stop=True)
            nc.scalar.copy(outT_sb[:, n0:n0 + w], ps_o[:, :w])

        def body():
            binning()
            nc.sync.dma_start(out_d[:, :], outT_sb[:])

        if reps == 1:
            body()
        else:
            with tc.For_i(0, reps, 1):
                body()

    nc.compile()
    return nc


# ---------------------------------------------------------------------------
# PJRT runner with device-resident inputs + repeat timing
# ---------------------------------------------------------------------------

def _run_spmd_pjrt(nc, in_maps, n_cores, timing_iters=0):
    import time as _time

    import jax
    from jax.experimental.shard_map import shard_map
    from jax.sharding import Mesh, NamedSharding, PartitionSpec

    from concourse import bass2jax, mybir

    bass2jax.install_neuronx_cc_hook()
    partition_name = (nc.partition_id_tensor.name
                      if nc.partition_id_tensor else None)
    in_names, out_names, out_avals, zero_outs = [], [], [], []
    for alloc in nc.m.functions[0].allocations:
        if not isinstance(alloc, mybir.MemoryLocationSet):
            continue
        name = alloc.memorylocations[0].name
        if alloc.kind == "ExternalInput":
            if name != partition_name:
                in_names.append(name)
        elif alloc.kind == "ExternalOutput":
            shape = tuple(alloc.tensor_shape)
            dtype = mybir.dt.np(alloc.dtype)
            out_names.append(name)
            out_avals.append(jax.core.ShapedArray(shape, dtype))
            zero_outs.append(np.zeros(shape, dtype))
    n_params = len(in_names)
    n_outs = len(out_avals)
    all_names = list(in_names) + list(out_names)
    if partition_name is not None:
        all_names.append(partition_name)
    donate = tuple(range(n_params, n_params + n_outs))

    def _body(*args):
        operands = list(args)
        if partition_name is not None:
            operands.append(bass2jax.partition_id_tensor())
        outs = bass2jax._bass_exec_p.bind(
            *operands,
            out_avals=tuple(out_avals),
            in_names=tuple(all_names),
            out_names=tuple(out_names),
            lowering_input_output_aliases=(),
            sim_require_finite=True,
            sim_require_nnan=True,
            nc=nc,
        )
        return tuple(outs)

    devices = jax.devices()[:n_cores]
    mesh = Mesh(np.asarray(devices), ("core",))
    in_specs = (PartitionSpec("core"),) * (n_params + n_outs)
    out_specs = (PartitionSpec("core"),) * len(out_names)
    fn = jax.jit(
        shard_map(_body, mesh=mesh, in_specs=in_specs, out_specs=out_specs,
                  check_rep=False),
        donate_argnums=donate, keep_unused=True)
    sharding = NamedSharding(mesh, PartitionSpec("core"))
    concat_in = [
        jax.device_put(
            np.concatenate([np.asarray(in_maps[c][nm]) for c in range(n_cores)],
                           axis=0), sharding)
        for nm in in_names
    ]

    def zeros():
        zs = [jax.device_put(
            np.zeros((n_cores * z.shape[0], *z.shape[1:]), z.dtype), sharding)
            for z in zero_outs]
        for z in zs:
            z.block_until_ready()
        return zs

    out_arrs = fn(*concat_in, *zeros())
    for o in out_arrs:
        o.block_until_ready()
    times = []
    for _ in range(timing_iters):
        zs = zeros()
        t0 = _time.perf_counter()
        outs2 = fn(*concat_in, *zs)
        for o in outs2:
            o.block_until_ready()
        times.append(_time.perf_counter() - t0)
    results = [
        {name: np.asarray(out_arrs[i]).reshape(n_cores, *out_avals[i].shape)[c]
         for i, name in enumerate(out_names)}
        for c in range(n_cores)
    ]
    return results, times


# ---------------------------------------------------------------------------
# Entry point
# ---------------------------------------------------------------------------

_BUILD_CACHE = {}


def make_in_maps(x_sh, rbf_sh, r_sh, W_rbf, W_up, W_mlp, b_mlp, W_final):
    W_rbf = np.ascontiguousarray(W_rbf, dtype=np.float32).astype(BF16)
    # fold the bias-free up-projection into the first MLP layer (fp64 host
    # precompute): h1 = silu(pooled @ (W_up @ W_mlp[0]) + b0)
    W_up = (np.asarray(W_up, np.float64) @ np.asarray(W_mlp[0], np.float64)
            ).astype(np.float32)
    W_mlp = np.ascontiguousarray(W_mlp, dtype=np.float32)
    W_final = np.ascontiguousarray(W_final, dtype=np.float32)
    b_mlp = np.asarray(b_mlp, dtype=np.float32)
    NL = W_mlp.shape[0]
    b_h = np.zeros((P, 2 * NL), dtype=np.float32)
    for i in range(NL):
        for oh in range(2):
            b_h[:, 2 * i + oh] = b_mlp[i, oh * P:(oh + 1) * P]
    iota_h = np.broadcast_to(
        np.arange(P, dtype=np.float32)[None, :], (P, P)).astype(BF16)
    ident_h = np.eye(P, dtype=np.float32).astype(BF16)

    in_maps = []
    for c in range(N_CORES):
        in_maps.append({
            "x_sh": x_sh[c],
            "rbf_sh": rbf_sh[c],
            "r_sh": r_sh[c],
            "W_rbf": W_rbf,
            "W_up": W_up,
            "W_mlp": W_mlp,
            "b_h": b_h,
            "W_final": W_final,
            "iota_h": iota_h,
            "ident_h": ident_h,
        })
    return in_maps


def kernel(n_atoms, x, rbf, idnb_i, W_rbf, W_up, W_mlp, b_mlp, W_final,
           timing_iters=0, reps=1, run_kwargs=None):
    n_nodes = n_atoms.shape[0]
    x_sh, rbf_sh, r_sh, meta = prepare_inputs(x, rbf, idnb_i, n_nodes)

    key = (n_nodes, tuple(meta["chunks"]), tuple(meta["ident"]), reps)
    if key not in _BUILD_CACHE:
        _BUILD_CACHE[key] = build(meta, reps=reps)
    nc = _BUILD_CACHE[key]

    in_maps = make_in_maps(x_sh, rbf_sh, r_sh, W_rbf, W_up, W_mlp, b_mlp,
                           W_final)
    try:
        results, times = _run_spmd_pjrt(nc, in_maps, N_CORES,
                                        timing_iters=timing_iters)
    except Exception:
        from concourse.bass_utils import run_bass_kernel_spmd
        res = run_bass_kernel_spmd(nc, in_maps, core_ids=list(range(N_CORES)))
        results, times = res.results, []
    asgn = np.asarray(meta["asgn"])
    n_tiles_total = _ceil_div(n_nodes, P)
    full = np.zeros((asgn.max() + 1) * P * NUM_TARGETS, np.float32).reshape(
        -1, NUM_TARGETS)
    for c in range(N_CORES):
        outc = results[c]["outT"].T          # [slots*P, 12]
        for t in range(meta["tiles_per_core"]):
            g = int(asgn[c, t])
            if g < n_tiles_total:
                full[g * P:(g + 1) * P] = outc[t * P:(t + 1) * P]
    full = full[:n_nodes]
    kernel.last_times = times
    return full.astype(np.float32)
